# revision 23
# baseline (speedup 1.0000x reference)
"""Trainium2 Bass kernel for CycleBalanceLoss.

loss = ALPHA * mean_b |sum_l adj[b, argmax_l, argmax_{l+1}]|
     + (1-ALPHA) * mean_{b,l} (logsumexp(logits[b,l,:]) - logits[b,l,t[b,l]])

Sharding: pure data parallel over the batch dim B=64 across 8 cores
(BPC=8 batches per core). Host sums the 8 per-core partial scalars.

Per core:
  - stream the logits shard [8, 128, 1024] f32 through SBUF;
  - ScalarE computes exp(x) with a per-row f32 accumulator (-> logsumexp),
    writing the exp values as fp16: argmax(exp(x)) == argmax(x), so the
    DVE max/max_index pass runs on 2-byte data;
  - target logits are gathered with per-column HW-DGE indirect DMAs
    (host-precomputed flat indices) issued before the loop so they overlap
    the stream;
  - the adjacency pair index pair[l] = 1024*idx[l] + idx[l+1] is ONE tiny
    PE matmul per batch against PAIRM = 1024*I + subdiag(1) (the subdiag
    also does the l+1 partition shift and zeroes the pad row), then a
    cast to u32 feeds a per-batch indirect DMA issued right after that
    batch's argmax, so gathers pace with the loop and only the last
    batch's gather (~2.7us + transfer) sits in the tail;
  - tc.tile_wait_until phases pin per-engine queue order so the tile
    scheduler cannot hoist gather-dependent ops ahead of the argmax work
    (its DMA cost model is optimistic, which otherwise stalls the
    in-order queues).
"""

import numpy as np

B, L, N = 64, 128, 1024
NCORES = 8
BPC = B // NCORES
ALPHA = 0.7

_CACHE = {}


def _build():
    import concourse.bacc as bacc
    import concourse.tile as tile
    from concourse import bass, mybir

    f32 = mybir.dt.float32
    fp16 = mybir.dt.float16
    i32 = mybir.dt.int32
    u16 = mybir.dt.uint16
    u32 = mybir.dt.uint32
    AF = mybir.ActivationFunctionType
    Alu = mybir.AluOpType
    AX = mybir.AxisListType

    nc = bacc.Bacc(
        "TRN2",
        target_bir_lowering=False,
        debug=False,
        num_devices=NCORES,
    )

    logits = nc.dram_tensor("logits", [BPC, L, N], f32, kind="ExternalInput")
    tfidx = nc.dram_tensor("tfidx", [L, BPC], i32, kind="ExternalInput")
    adj = nc.dram_tensor("adj", [BPC * N * N, 1], f32, kind="ExternalInput")
    pairm = nc.dram_tensor("pairm", [128, 128], f32, kind="ExternalInput")
    out = nc.dram_tensor("out", [2, 1], f32, kind="ExternalOutput")

    logits_ap = logits.ap()
    logits_flat = logits_ap.rearrange("b l n -> (b l n)")[:, None]

    with tile.TileContext(nc) as tc:
        with (
            tc.tile_pool(name="xp", bufs=4) as xp,
            tc.tile_pool(name="ep", bufs=3) as ep,
            tc.tile_pool(name="sp", bufs=3) as sp,
            tc.tile_pool(name="acc", bufs=1) as accp,
            tc.tile_pool(name="psum", bufs=2, space="PSUM") as pp,
        ):
            ones = accp.tile([L, 1], f32)
            nc.vector.memset(ones[:], 1.0)

            PM = accp.tile([128, 128], f32)
            nc.scalar.dma_start(PM[:], pairm.ap())

            # target flat indices (host precomputed) and target-logit gather.
            # NOTE: multi-column offset tables wedge the HW DGE - one
            # indirect DMA per column ([P,1] offsets) is the proven shape.
            TF = accp.tile([L, BPC], i32)
            nc.scalar.dma_start(TF[:], tfidx.ap())
            XT = accp.tile([L, BPC], f32)

            def xt_gather(b):
                nc.gpsimd.indirect_dma_start(
                    out=XT[:, b : b + 1],
                    out_offset=None,
                    in_=logits_flat,
                    in_offset=bass.IndirectOffsetOnAxis(ap=TF[:, b : b + 1], axis=0),
                )

            # cols 0-5 fill the Pool queue before the first adj gather is
            # ready; cols 6-7 go after the last adj gather (they are only
            # needed for the CE sum late in the kernel)
            for b in range(6):
                xt_gather(b)

            S = accp.tile([L, BPC], f32)
            M8 = accp.tile([L, BPC * 8], fp16)
            W = accp.tile([L, BPC], f32)  # rows 0..126 hold path weights

            def batch(b):
                X = xp.tile([L, N], f32, tag="X")
                nc.sync.dma_start(X[:], logits_ap[b])
                E = ep.tile([L, N], fp16, tag="E")
                nc.scalar.activation(E[:], X[:], AF.Exp, accum_out=S[:, b : b + 1])
                nc.vector.max(M8[:, 8 * b : 8 * b + 8], E[:])
                i8 = sp.tile([L, 8], u16, tag="i8")
                nc.vector.max_index(i8[:], M8[:, 8 * b : 8 * b + 8], E[:])

                # pair[l] = 1024*idx[l] + idx[l+1] via one PE matmul
                idxf = sp.tile([L, 1], f32, tag="idxf")
                nc.vector.tensor_copy(idxf[:], i8[:, 0:1])
                pairp = pp.tile([L, 1], f32)
                nc.tensor.matmul(
                    out=pairp[:], lhsT=PM[:], rhs=idxf[:], start=True, stop=True
                )
                pairu = sp.tile([L, 1], u32, tag="pairu")
                nc.vector.tensor_copy(pairu[0 : L - 1, :], pairp[0 : L - 1, :])
                nc.gpsimd.indirect_dma_start(
                    out=W[0 : L - 1, b : b + 1],
                    out_offset=None,
                    in_=adj.ap(),
                    in_offset=bass.IndirectOffsetOnAxis(
                        ap=pairu[0 : L - 1, :], axis=0
                    ),
                    element_offset=b * N * N,
                )

            for b in range(BPC):
                with tc.tile_wait_until(0.004 * b):
                    batch(b)

            with tc.tile_wait_until(0.034):
                xt_gather(6)
                xt_gather(7)

            with tc.tile_wait_until(0.036):
                # cross-entropy partial: sum(ln S - x_t)
                LSE = accp.tile([L, BPC], f32)
                nc.scalar.activation(LSE[:], S[:], AF.Ln)
                R = accp.tile([L, 2], f32)
                nc.vector.memset(R[:, 1:2], 0.0)
                NLL = accp.tile([L, BPC], f32)
                nc.vector.tensor_sub(NLL[:], LSE[:], XT[:])
                nc.vector.reduce_sum(R[:, 0:1], NLL[:], axis=AX.X)

            with tc.tile_wait_until(0.038):
                # balance partial: |per-batch path sums| via PE
                ps_b = pp.tile([BPC, 1], f32)
                nc.tensor.matmul(
                    out=ps_b[:], lhsT=W[0 : L - 1, :], rhs=ones[0 : L - 1, :],
                    start=True, stop=True,
                )
                nc.scalar.activation(R[0:BPC, 1:2], ps_b[:], AF.Abs)

                ps2 = pp.tile([2, 1], f32)
                nc.tensor.matmul(out=ps2[:], lhsT=R[:], rhs=ones[:], start=True, stop=True)
                c2 = sp.tile([2, 1], f32, tag="c2")
                nc.vector.tensor_copy(c2[:], ps2[:])
                nc.scalar.dma_start(out.ap(), c2[:])

    nc.compile()
    return nc


def _get_nc():
    if "nc" not in _CACHE:
        _CACHE["nc"] = _build()
    return _CACHE["nc"]


def _consts():
    if "consts" in _CACHE:
        return _CACHE["consts"]
    ls = np.arange(128)
    pairmm = 1024.0 * (ls[:, None] == ls[None, :]).astype(np.float32) + (
        ls[:, None] == ls[None, :] + 1
    ).astype(np.float32)
    _CACHE["consts"] = pairmm
    return _CACHE["consts"]


def make_in_maps(path_logits, target_paths, adj_matrix):
    """Shard full inputs into per-core in_maps (host-side packing only)."""
    pairmm = _consts()
    l_off = np.arange(L, dtype=np.int64) * N
    b_off = np.arange(BPC, dtype=np.int64)[:, None] * (L * N)
    in_maps = []
    for c in range(NCORES):
        sl = slice(c * BPC, (c + 1) * BPC)
        lg = np.ascontiguousarray(path_logits[sl], dtype=np.float32)
        ad = np.ascontiguousarray(adj_matrix[sl], dtype=np.float32).reshape(
            BPC * N * N, 1
        )
        t = np.asarray(target_paths[sl], dtype=np.int64)
        tf = (b_off + l_off[None, :] + t).astype(np.int32)
        in_maps.append(
            {
                "logits": lg,
                "tfidx": np.ascontiguousarray(tf.T),
                "adj": ad,
                "pairm": pairmm,
            }
        )
    return in_maps


def kernel(**inputs):
    from concourse import bass_utils

    nc = _get_nc()
    in_maps = make_in_maps(
        inputs["path_logits"], inputs["target_paths"], inputs["adj_matrix"]
    )
    res = bass_utils.run_bass_kernel_spmd(nc, in_maps, core_ids=list(range(NCORES)))
    w_nll = np.float32((1.0 - ALPHA) / (B * L))
    w_bal = np.float32(ALPHA / B)
    total = np.float32(0.0)
    for r in res.results:
        total = total + w_nll * np.float32(r["out"][0, 0]) + w_bal * np.float32(
            r["out"][1, 0]
        )
    return np.asarray(total, dtype=np.float32)


# revision 24
# speedup vs baseline: 1.0231x; 1.0231x over previous
"""Trainium2 Bass kernel for CycleBalanceLoss.

loss = ALPHA * mean_b |sum_l adj[b, argmax_l, argmax_{l+1}]|
     + (1-ALPHA) * mean_{b,l} (logsumexp(logits[b,l,:]) - logits[b,l,t[b,l]])

Sharding: pure data parallel over the batch dim B=64 across 8 cores
(BPC=8 batches per core). Host sums the 8 per-core partial scalars.

Per core:
  - stream the logits shard [8, 128, 1024] f32 through SBUF;
  - ScalarE computes exp(x) with a per-row f32 accumulator (-> logsumexp),
    writing the exp values as fp16: argmax(exp(x)) == argmax(x), so the
    DVE max/max_index pass runs on 2-byte data;
  - target logits are gathered with per-column HW-DGE indirect DMAs
    (host-precomputed flat indices) issued before the loop so they overlap
    the stream;
  - the adjacency pair index pair[l] = 1024*idx[l] + idx[l+1] is ONE tiny
    PE matmul per batch against PAIRM = 1024*I + subdiag(1) (the subdiag
    also does the l+1 partition shift and zeroes the pad row), then a
    cast to u32 feeds a per-batch indirect DMA issued right after that
    batch's argmax, so gathers pace with the loop and only the last
    batch's gather (~2.7us + transfer) sits in the tail;
  - tc.tile_wait_until phases pin per-engine queue order so the tile
    scheduler cannot hoist gather-dependent ops ahead of the argmax work
    (its DMA cost model is optimistic, which otherwise stalls the
    in-order queues).
"""

import numpy as np

B, L, N = 64, 128, 1024
NCORES = 8
BPC = B // NCORES
ALPHA = 0.7

_CACHE = {}


def _build():
    import concourse.bacc as bacc
    import concourse.tile as tile
    from concourse import bass, mybir

    f32 = mybir.dt.float32
    fp16 = mybir.dt.float16
    i32 = mybir.dt.int32
    u16 = mybir.dt.uint16
    u32 = mybir.dt.uint32
    AF = mybir.ActivationFunctionType
    Alu = mybir.AluOpType
    AX = mybir.AxisListType

    nc = bacc.Bacc(
        "TRN2",
        target_bir_lowering=False,
        debug=False,
        num_devices=NCORES,
    )

    logits = nc.dram_tensor("logits", [BPC, L, N], f32, kind="ExternalInput")
    tfidx = nc.dram_tensor("tfidx", [L, BPC], i32, kind="ExternalInput")
    adj = nc.dram_tensor("adj", [BPC * N * N, 1], f32, kind="ExternalInput")
    pairm = nc.dram_tensor("pairm", [128, 128], f32, kind="ExternalInput")
    out = nc.dram_tensor("out", [2, 1], f32, kind="ExternalOutput")

    logits_ap = logits.ap()
    logits_flat = logits_ap.rearrange("b l n -> (b l n)")[:, None]

    with tile.TileContext(nc) as tc:
        with (
            tc.tile_pool(name="xp", bufs=4) as xp,
            tc.tile_pool(name="ep", bufs=3) as ep,
            tc.tile_pool(name="sp", bufs=3) as sp,
            tc.tile_pool(name="acc", bufs=1) as accp,
            tc.tile_pool(name="psum", bufs=2, space="PSUM") as pp,
        ):
            ones = accp.tile([L, 1], f32)
            nc.vector.memset(ones[:], 1.0)

            PM = accp.tile([128, 128], f32)
            nc.scalar.dma_start(PM[:], pairm.ap())

            # target flat indices (host precomputed) and target-logit gather.
            # NOTE: multi-column offset tables wedge the HW DGE - one
            # indirect DMA per column ([P,1] offsets) is the proven shape.
            TF = accp.tile([L, BPC], i32)
            nc.scalar.dma_start(TF[:], tfidx.ap())
            XT = accp.tile([L, BPC], f32)

            def xt_gather(b):
                nc.gpsimd.indirect_dma_start(
                    out=XT[:, b : b + 1],
                    out_offset=None,
                    in_=logits_flat,
                    in_offset=bass.IndirectOffsetOnAxis(ap=TF[:, b : b + 1], axis=0),
                )

            # cols 0-5 fill the Pool queue before the first adj gather is
            # ready; cols 6-7 go after the last adj gather (they are only
            # needed for the CE sum late in the kernel)
            for b in range(6):
                xt_gather(b)

            S = accp.tile([L, BPC], f32)
            M8 = accp.tile([L, BPC * 8], fp16)
            W = accp.tile([L, BPC], f32)  # rows 0..126 hold path weights

            def batch(b):
                X = xp.tile([L, N], f32, tag="X")
                nc.sync.dma_start(X[:], logits_ap[b])
                E = ep.tile([L, N], fp16, tag="E")
                nc.scalar.activation(E[:], X[:], AF.Exp, accum_out=S[:, b : b + 1])
                nc.vector.max(M8[:, 8 * b : 8 * b + 8], E[:])
                i8 = sp.tile([L, 8], u16, tag="i8")
                nc.vector.max_index(i8[:], M8[:, 8 * b : 8 * b + 8], E[:])

                # pair[l] = 1024*idx[l] + idx[l+1] via one PE matmul
                idxf = sp.tile([L, 1], f32, tag="idxf")
                nc.vector.tensor_copy(idxf[:], i8[:, 0:1])
                pairp = pp.tile([L, 1], f32)
                nc.tensor.matmul(
                    out=pairp[:], lhsT=PM[:], rhs=idxf[:], start=True, stop=True
                )
                pairu = sp.tile([L, 1], i32, tag="pairu")
                nc.vector.tensor_scalar(
                    pairu[0 : L - 1, :], pairp[0 : L - 1, :], float(b * N * N),
                    None, op0=Alu.add,
                )
                nc.gpsimd.indirect_dma_start(
                    out=W[0 : L - 1, b : b + 1],
                    out_offset=None,
                    in_=adj.ap(),
                    in_offset=bass.IndirectOffsetOnAxis(
                        ap=pairu[0 : L - 1, :], axis=0
                    ),
                )

            for b in range(BPC):
                with tc.tile_wait_until(0.0024 * b):
                    batch(b)

            with tc.tile_wait_until(0.0185):
                xt_gather(6)
                xt_gather(7)

            with tc.tile_wait_until(0.0195):
                # cross-entropy partial: sum(ln S - x_t)
                LSE = accp.tile([L, BPC], f32)
                nc.scalar.activation(LSE[:], S[:], AF.Ln)
                R = accp.tile([L, 2], f32)
                nc.vector.memset(R[:, 1:2], 0.0)
                NLL = accp.tile([L, BPC], f32)
                nc.vector.tensor_sub(NLL[:], LSE[:], XT[:])
                nc.vector.reduce_sum(R[:, 0:1], NLL[:], axis=AX.X)

            with tc.tile_wait_until(0.021):
                # balance partial: |per-batch path sums| via PE
                ps_b = pp.tile([BPC, 1], f32)
                nc.tensor.matmul(
                    out=ps_b[:], lhsT=W[0 : L - 1, :], rhs=ones[0 : L - 1, :],
                    start=True, stop=True,
                )
                nc.scalar.activation(R[0:BPC, 1:2], ps_b[:], AF.Abs)

                ps2 = pp.tile([2, 1], f32)
                nc.tensor.matmul(out=ps2[:], lhsT=R[:], rhs=ones[:], start=True, stop=True)
                c2 = sp.tile([2, 1], f32, tag="c2")
                nc.vector.tensor_copy(c2[:], ps2[:])
                nc.scalar.dma_start(out.ap(), c2[:])

    nc.compile()
    return nc


def _get_nc():
    if "nc" not in _CACHE:
        _CACHE["nc"] = _build()
    return _CACHE["nc"]


def _consts():
    if "consts" in _CACHE:
        return _CACHE["consts"]
    ls = np.arange(128)
    pairmm = 1024.0 * (ls[:, None] == ls[None, :]).astype(np.float32) + (
        ls[:, None] == ls[None, :] + 1
    ).astype(np.float32)
    _CACHE["consts"] = pairmm
    return _CACHE["consts"]


def make_in_maps(path_logits, target_paths, adj_matrix):
    """Shard full inputs into per-core in_maps (host-side packing only)."""
    pairmm = _consts()
    l_off = np.arange(L, dtype=np.int64) * N
    b_off = np.arange(BPC, dtype=np.int64)[:, None] * (L * N)
    in_maps = []
    for c in range(NCORES):
        sl = slice(c * BPC, (c + 1) * BPC)
        lg = np.ascontiguousarray(path_logits[sl], dtype=np.float32)
        ad = np.ascontiguousarray(adj_matrix[sl], dtype=np.float32).reshape(
            BPC * N * N, 1
        )
        t = np.asarray(target_paths[sl], dtype=np.int64)
        tf = (b_off + l_off[None, :] + t).astype(np.int32)
        in_maps.append(
            {
                "logits": lg,
                "tfidx": np.ascontiguousarray(tf.T),
                "adj": ad,
                "pairm": pairmm,
            }
        )
    return in_maps


def kernel(**inputs):
    from concourse import bass_utils

    nc = _get_nc()
    in_maps = make_in_maps(
        inputs["path_logits"], inputs["target_paths"], inputs["adj_matrix"]
    )
    res = bass_utils.run_bass_kernel_spmd(nc, in_maps, core_ids=list(range(NCORES)))
    w_nll = np.float32((1.0 - ALPHA) / (B * L))
    w_bal = np.float32(ALPHA / B)
    total = np.float32(0.0)
    for r in res.results:
        total = total + w_nll * np.float32(r["out"][0, 0]) + w_bal * np.float32(
            r["out"][1, 0]
        )
    return np.asarray(total, dtype=np.float32)
